# revision 7
# baseline (speedup 1.0000x reference)
"""Trainium2 Bass kernel for nn_CSFM_86011015070100 (topk_masking).

Data-parallel over batch: core b handles batch element b (B == 8 == n_cores).

Two launches per batch element:
  L1 (fused, rgb SBUF-resident):
    phase 1: stream rgb (kept resident) + ir; per-pixel channel-sum and
        channel-max via pair-op + PE transpose + DVE reduce; per-channel
        sum(x^2) partials on ScalarE.
    phase 2: 7x7 conv on device as 14 accumulating PE matmuls against
        host-precomputed banded weight matrices; double sigmoid via a
        degree-9 f32 exp polynomial + DVE reciprocal (~1 ulp).
    phase 3: per-channel dot(sa, x_c) partial sums (16-px blocks); rgb read
        from SBUF, ir re-streamed; sa broadcast to 128 partitions by a
        bit-exact fp32 ones-matmul on the otherwise idle PE.
  host: combine partials in f64 (exact sim ordering), stable argsort,
    positive counts, global k, gather tables.
  L2: indirect-DMA channel gather of rgb/ir + add -> output; host fixes up
    the single max-fused channel (when k_rgb != k_ir).
"""

import numpy as np
from contextlib import ExitStack

import concourse.bass as bass
import concourse.bacc as bacc
import concourse.tile as tile
from concourse import mybir
from concourse.bass_utils import run_bass_kernel_spmd
from concourse.masks import make_identity

F32 = mybir.dt.float32
I32 = mybir.dt.int32

B, C, H, W = 8, 256, 128, 128
HW = H * W          # 16384
NCORES = 8
CORE_IDS = list(range(NCORES))

PCH = 1024          # pixels per chunk in launch 1 (both phases)
NCH = HW // PCH     # 16
DBLK = 16           # pixels per dot partial-sum block
NDP = HW // DBLK    # 1024 dot partials per channel
SBLK = PCH          # pixels per sum-of-squares partial block
NSP = HW // SBLK    # 16
GCHUNK = 4096       # pixels per gather chunk in L2
NGCH = HW // GCHUNK

# degree-9 minimax fit of exp(x) on [-1.5, 1.0]; ~5e-8 rel fit error
EXPC = (0.999999998645628, 0.9999999779969408, 0.5000000450179761,
        0.16666694288780706, 0.04166648306345862, 0.008332373827203384,
        0.001388966966072117, 0.00019966415132577638,
        2.5136363775979867e-05, 2.2273871202240795e-06)

_cache = {}

TRACE = False
LAST_EXEC_NS = []


def _run(nc, maps):
    try:
        r = run_bass_kernel_spmd(nc, maps, CORE_IDS, trace=TRACE)
    except Exception:
        import time

        time.sleep(2)
        r = run_bass_kernel_spmd(nc, maps, CORE_IDS, trace=TRACE)
    if r.exec_time_ns is not None:
        LAST_EXEC_NS.append(r.exec_time_ns)
    return r.results


# --------------------------------------------------------------------------
# L1: stats + on-device spatial attention + dot partials
# --------------------------------------------------------------------------
def _sigmoid_dev(nc, out, m, scratch):
    """out = 1/(1+exp(-m)) elementwise on a [128,128] tile, ~1 ulp f32."""
    vec = nc.vector
    u = scratch()
    t = scratch()
    acc = scratch()
    vec.tensor_scalar(out=u[:], in0=m, scalar1=-1.0, scalar2=None,
                      op0=mybir.AluOpType.mult)
    vec.memset(acc[:], float(EXPC[9]))
    for i in range(8, -1, -1):
        vec.tensor_tensor(out=t[:], in0=acc[:], in1=u[:],
                          op=mybir.AluOpType.mult)
        vec.tensor_scalar(out=acc[:], in0=t[:], scalar1=float(EXPC[i]),
                          scalar2=None, op0=mybir.AluOpType.add)
    vec.tensor_scalar(out=t[:], in0=acc[:], scalar1=1.0, scalar2=None,
                      op0=mybir.AluOpType.add)
    vec.reciprocal(out=out, in_=t[:])


def _build_l1():
    nc = bacc.Bacc("TRN2", target_bir_lowering=False, debug=False)
    rgb = nc.dram_tensor("rgb", [C, HW], F32, kind="ExternalInput").ap()
    ir = nc.dram_tensor("ir", [C, HW], F32, kind="ExternalInput").ap()
    # banded conv weights [128 w_in, 14 (plane*7+dh), 128 w_out]; avg-plane
    # bands pre-scaled by 1/256 on host
    wband = nc.dram_tensor("wband", [128, 14, 128], F32,
                           kind="ExternalInput").ap()
    bvec = nc.dram_tensor("bvec", [1, 1], F32, kind="ExternalInput").ap()
    dparts = nc.dram_tensor("dparts", [2, 2, 128, NDP], F32,
                            kind="ExternalOutput").ap()
    sparts = nc.dram_tensor("sparts", [2, 2, 128, NSP], F32,
                            kind="ExternalOutput").ap()
    sad = nc.dram_tensor("sad", [128, 128], F32, kind="Internal").ap()

    with tile.TileContext(nc) as tc, ExitStack() as ctx:
        consts = ctx.enter_context(tc.tile_pool(name="consts", bufs=1))
        ident = consts.tile([128, 128], F32, name="ident")
        make_identity(nc, ident[:])
        ones1 = consts.tile([1, 128], F32, name="ones1")
        nc.vector.memset(ones1[:], 1.0)
        # resident rgb: [128, 2 groups, 16384] = 128 KiB/partition
        rgbres = consts.tile([128, 2, HW], F32, name="rgbres")
        # stats planes [w, h] with 3-col zero padding each side (h -> 134)
        swh = {}
        for t in range(2):
            for kind in ("s", "m"):
                p = consts.tile([128, 134], F32, name=f"swh_{kind}{t}")
                nc.vector.memset(p[:], 0.0)
                swh[kind, t] = p
        sp_acc = {}
        dp_acc = {}
        for t in range(2):
            for g in range(2):
                sp_acc[t, g] = consts.tile([128, NSP], F32, name=f"sp{t}{g}")
                dp_acc[t, g] = consts.tile([128, NDP], F32, name=f"dp{t}{g}")
        bsb = consts.tile([128, 1], F32, name="bsb")
        b_bcast = bass.AP(tensor=bvec.tensor, offset=bvec.offset,
                          ap=[[0, 128], [1, 1]])
        nc.sync.dma_start(out=bsb[:], in_=b_bcast)

        # ---------------- phase 1: stats ----------------
        with tc.tile_pool(name="ld1", bufs=2) as ld1, \
             tc.tile_pool(name="pair", bufs=2) as pairp, \
             tc.tile_pool(name="sqd", bufs=2) as sqd, \
             tc.tile_pool(name="pt1", bufs=2, space="PSUM") as pt1:
            for ci in range(NCH):
                sl = slice(ci * PCH, (ci + 1) * PCH)
                for t, x in enumerate((rgb, ir)):
                    if t == 0:
                        nc.sync.dma_start(out=rgbres[:, 0, sl], in_=x[0:128, sl])
                        nc.sync.dma_start(out=rgbres[:, 1, sl], in_=x[128:256, sl])
                        x0 = rgbres[:, 0, sl]
                        x1 = rgbres[:, 1, sl]
                    else:
                        xt0 = ld1.tile([128, PCH], F32, tag="ir0")
                        xt1 = ld1.tile([128, PCH], F32, tag="ir1")
                        nc.sync.dma_start(out=xt0[:], in_=x[0:128, sl])
                        nc.sync.dma_start(out=xt1[:], in_=x[128:256, sl])
                        x0 = xt0[:]
                        x1 = xt1[:]
                    # sumsq partials (ScalarE, full-chunk accumulate)
                    for g, xg in enumerate((x0, x1)):
                        sq = sqd.tile([128, PCH], F32, tag="sq")
                        nc.scalar.activation(
                            out=sq[:], in_=xg,
                            func=mybir.ActivationFunctionType.Square,
                            accum_out=sp_acc[t, g][:, ci:ci + 1])
                    # pair ops
                    gsum = pairp.tile([128, PCH], F32, tag="gsum")
                    nc.gpsimd.tensor_tensor(out=gsum[:], in0=x0, in1=x1,
                                            op=mybir.AluOpType.add)
                    gm = pairp.tile([128, PCH], F32, tag="gm")
                    nc.vector.tensor_tensor(out=gm[:], in0=x0, in1=x1,
                                            op=mybir.AluOpType.max)
                    # transpose 8 blocks of each into PSUM, then one reduce
                    for kind, src, op in (("s", gsum, mybir.AluOpType.add),
                                          ("m", gm, mybir.AluOpType.max)):
                        pt = pt1.tile([128, 8, 128], F32, tag=f"pt{kind}")
                        for b8 in range(8):
                            nc.tensor.transpose(
                                pt[:, b8], src[:, b8 * 128:(b8 + 1) * 128],
                                ident[:])
                        nc.vector.tensor_reduce(
                            out=swh[kind, t][:, 3 + ci * 8:3 + ci * 8 + 8],
                            in_=pt[:], axis=mybir.AxisListType.X, op=op)

        # ---------------- phase 2: sa on device ----------------
        with tc.tile_pool(name="sas", bufs=1) as sas, \
             tc.tile_pool(name="pt2", bufs=1, space="PSUM") as pt2:
            wb = sas.tile([128, 14, 128], F32, name="wb")
            nc.sync.dma_start(out=wb[:], in_=wband)
            _sc = [0]

            def scratch():
                _sc[0] += 1
                return sas.tile([128, 128], F32, name=f"scr{_sc[0]}")

            convs = []
            for t in range(2):
                cp = pt2.tile([128, 128], F32, tag=f"conv{t}")
                idx = 0
                for plane, kind in ((0, "s"), (1, "m")):
                    for dh in range(7):
                        nc.tensor.matmul(
                            cp[:], wb[:, plane * 7 + dh],
                            swh[kind, t][:, dh:dh + 128],
                            start=(idx == 0), stop=(idx == 13))
                        idx += 1
                cv = sas.tile([128, 128], F32, name=f"cv{t}")
                nc.scalar.copy(out=cv[:], in_=cp[:])
                convs.append(cv)
            m = sas.tile([128, 128], F32, name="mtile")
            nc.vector.tensor_tensor(out=m[:], in0=convs[0][:], in1=convs[1][:],
                                    op=mybir.AluOpType.max)
            nc.vector.tensor_scalar(out=m[:], in0=m[:], scalar1=bsb[:, 0:1],
                                    scalar2=None, op0=mybir.AluOpType.add)
            s1 = sas.tile([128, 128], F32, name="s1")
            _sigmoid_dev(nc, s1[:], m[:], scratch)
            sa_wh = sas.tile([128, 128], F32, name="sa_wh")
            _sigmoid_dev(nc, sa_wh[:], s1[:], scratch)
            # transpose [w,h] -> [h,w]; round-trip through DRAM to reshape
            # into [16 chunks, 1024] for the phase-3 broadcast matmuls
            pt = pt2.tile([128, 128], F32, tag="satp")
            nc.tensor.transpose(pt[:], sa_wh[:], ident[:])
            sa_hw = sas.tile([128, 128], F32, name="sa_hw")
            nc.scalar.copy(out=sa_hw[:], in_=pt[:])
            nc.scalar.dma_start(out=sad, in_=sa_hw[:])

        # ---------------- phase 3: dot partials ----------------
        with tc.tile_pool(name="ld3", bufs=2) as ld3, \
             tc.tile_pool(name="sab", bufs=2) as sab, \
             tc.tile_pool(name="prodp", bufs=2) as prodp, \
             tc.tile_pool(name="pt3", bufs=2, space="PSUM") as pt3, \
             tc.tile_pool(name="sarow", bufs=2) as sarow:
            for ci in range(NCH):
                sl = slice(ci * PCH, (ci + 1) * PCH)
                # broadcast sa chunk to 128 partitions: exact fp32 ones-matmul
                srow = sarow.tile([1, PCH], F32, tag="srow")
                sa_rd = bass.AP(tensor=sad.tensor, offset=ci * PCH,
                                ap=[[PCH, 1], [1, PCH]])
                nc.scalar.dma_start(out=srow[:], in_=sa_rd)
                ptb = pt3.tile([128, PCH], F32, tag="ptb")
                for k in range(PCH // 512):
                    nc.tensor.matmul(ptb[:, k * 512:(k + 1) * 512], ones1[:],
                                     srow[:, k * 512:(k + 1) * 512],
                                     start=True, stop=True)
                sa128 = sab.tile([128, PCH], F32, tag="sa128")
                nc.scalar.copy(out=sa128[:], in_=ptb[:])
                step = 0
                for t, x in enumerate((rgb, ir)):
                    for g in range(2):
                        if t == 0:
                            xg = rgbres[:, g, sl]
                        else:
                            xt = ld3.tile([128, PCH], F32, tag=f"ir3{g}")
                            nc.sync.dma_start(out=xt[:],
                                              in_=x[g * 128:(g + 1) * 128, sl])
                            xg = xt[:]
                        prod = prodp.tile([128, PCH], F32, tag="prod")
                        eng = nc.vector if step % 4 == 3 else nc.gpsimd
                        eng.tensor_tensor(out=prod[:], in0=xg, in1=sa128[:],
                                          op=mybir.AluOpType.mult)
                        nc.vector.tensor_reduce(
                            out=dp_acc[t, g][:, ci * (PCH // DBLK):(ci + 1) * (PCH // DBLK)],
                            in_=prod[:].rearrange("p (s q) -> p s q", q=DBLK),
                            axis=mybir.AxisListType.X, op=mybir.AluOpType.add)
                        step += 1
        for t in range(2):
            for g in range(2):
                nc.scalar.dma_start(out=dparts[t, g], in_=dp_acc[t, g][:])
                nc.scalar.dma_start(out=sparts[t, g], in_=sp_acc[t, g][:])

    nc.compile()
    return nc


# --------------------------------------------------------------------------
# L2: gather channels of rgb/ir by index and add
# --------------------------------------------------------------------------
def _build_l2():
    nc = bacc.Bacc("TRN2", target_bir_lowering=False, debug=False,
                   num_swdge_queues=2)
    rgb = nc.dram_tensor("rgb", [C, HW], F32, kind="ExternalInput").ap()
    ir = nc.dram_tensor("ir", [C, HW], F32, kind="ExternalInput").ap()
    gidx = nc.dram_tensor("gidx", [2, C], I32, kind="ExternalInput").ap()
    out = nc.dram_tensor("out", [C, HW], F32, kind="ExternalOutput").ap()

    with tile.TileContext(nc) as tc, ExitStack() as ctx:
        idxp = ctx.enter_context(tc.tile_pool(name="idxp", bufs=1))
        rp = ctx.enter_context(tc.tile_pool(name="rp", bufs=3))
        ip = ctx.enter_context(tc.tile_pool(name="ip", bufs=3))
        op = ctx.enter_context(tc.tile_pool(name="op", bufs=3))

        for g in range(2):
            idr = idxp.tile([128, 1], I32, tag=f"idr{g}")
            idi = idxp.tile([128, 1], I32, tag=f"idi{g}")
            nc.sync.dma_start(out=idr[:], in_=gidx[0, g * 128:(g + 1) * 128])
            nc.sync.dma_start(out=idi[:], in_=gidx[1, g * 128:(g + 1) * 128])
            for ci in range(NGCH):
                sl = slice(ci * GCHUNK, (ci + 1) * GCHUNK)
                rt = rp.tile([128, GCHUNK], F32, tag="rt")
                it = ip.tile([128, GCHUNK], F32, tag="it")
                nc.gpsimd.indirect_dma_start(
                    out=rt[:], out_offset=None, in_=rgb,
                    in_offset=bass.IndirectOffsetOnAxis(ap=idr[:, 0:1], axis=0),
                    element_offset=ci * GCHUNK)
                inst = nc.gpsimd.indirect_dma_start(
                    out=it[:], out_offset=None, in_=ir,
                    in_offset=bass.IndirectOffsetOnAxis(ap=idi[:, 0:1], axis=0),
                    element_offset=ci * GCHUNK)
                inst.ins.queue = "qPoolDynamic1"
                ot = op.tile([128, GCHUNK], F32, tag="ot")
                nc.vector.tensor_tensor(out=ot[:], in0=rt[:], in1=it[:],
                                        op=mybir.AluOpType.add)
                nc.sync.dma_start(out=out[g * 128:(g + 1) * 128, sl], in_=ot[:])

    nc.compile()
    return nc


def _get(name, builder):
    if name not in _cache:
        _cache[name] = builder()
    return _cache[name]


# --------------------------------------------------------------------------
# host glue
# --------------------------------------------------------------------------
def _make_wband(conv_w):
    """[128 w_in, 14 (plane*7+dh), 128 w_out]: W[wi, k, wo] = cw[plane,dh,dw]
    at wi = wo+dw-3; avg-plane entries pre-scaled by 1/256."""
    cw = np.asarray(conv_w, dtype=np.float64)  # [1,2,7,7]
    wb = np.zeros((14, 128, 128), np.float64)
    for plane in range(2):
        scale = (1.0 / 256.0) if plane == 0 else 1.0
        for dh in range(7):
            for dw in range(7):
                v = cw[0, plane, dh, dw] * scale
                wi0 = max(0, 3 - dw)
                wi = np.arange(128) + dw - 3
                ok = (wi >= 0) & (wi < 128)
                wb[plane * 7 + dh, wi[ok], np.arange(128)[ok]] = v
    return np.ascontiguousarray(wb.transpose(1, 0, 2)).astype(np.float32)


def kernel(rgb, ir, conv_w, conv_b):
    rgb = np.ascontiguousarray(rgb, dtype=np.float32)
    ir = np.ascontiguousarray(ir, dtype=np.float32)
    conv_w = np.asarray(conv_w, dtype=np.float32)
    conv_b = np.asarray(conv_b, dtype=np.float32)

    rgb2 = rgb.reshape(B, C, HW)
    ir2 = ir.reshape(B, C, HW)
    LAST_EXEC_NS.clear()

    wband = _make_wband(conv_w)
    bvec = conv_b.reshape(1, 1)

    # ---- L1
    nc1 = _get("l1", _build_l1)
    maps1 = [{"rgb": rgb2[b], "ir": ir2[b], "wband": wband, "bvec": bvec}
             for b in range(B)]
    res1 = _run(nc1, maps1)

    # ---- host: sims, orders, counts, tables (f64 combine of partials)
    orders = np.zeros((B, 2, C), np.int64)
    cnts = np.zeros((B, 2), np.int64)
    for b in range(B):
        dparts = res1[b]["dparts"].astype(np.float64)  # [2,2,128,NDP]
        sparts = res1[b]["sparts"].astype(np.float64)  # [2,2,128,NSP]
        for t in range(2):
            dot = np.concatenate([dparts[t, 0].sum(-1), dparts[t, 1].sum(-1)])
            sq = np.concatenate([sparts[t, 0].sum(-1), sparts[t, 1].sum(-1)])
            tv = dot / np.maximum(np.sqrt(sq), 1e-30)
            orders[b, t] = np.argsort(tv, kind="stable")
            cnts[b, t] = int((tv > 0).sum())
    k_rgb = int(cnts[:, 0].max())
    k_ir = int(cnts[:, 1].max())
    ch = np.arange(C)
    src_rgb = ch.copy()
    src_ir = ch.copy()
    if k_rgb < k_ir:
        src_rgb[ch > k_rgb] -= 1
    elif k_ir < k_rgb:
        src_ir[ch > k_ir] -= 1

    # ---- L2
    nc2 = _get("l2", _build_l2)
    gidxs = []
    for b in range(B):
        g_r = orders[b, 0][src_rgb]
        g_i = orders[b, 1][src_ir]
        gidxs.append(np.stack([g_r, g_i]).astype(np.int32))
    maps2 = [{"rgb": rgb2[b], "ir": ir2[b], "gidx": gidxs[b]} for b in range(B)]
    res2 = _run(nc2, maps2)
    out = np.stack([res2[b]["out"].reshape(C, H, W) for b in range(B)])

    # ---- host fixup of the max-fused channel
    if k_rgb != k_ir:
        kpos = min(k_rgb, k_ir)
        for b in range(B):
            maxfea = np.maximum(rgb2[b, orders[b, 0][0]], ir2[b, orders[b, 1][0]])
            if k_rgb < k_ir:
                other = ir2[b, gidxs[b][1][kpos]]
            else:
                other = rgb2[b, gidxs[b][0][kpos]]
            out[b, kpos] = (maxfea + other).reshape(H, W)

    return out


# revision 12
# speedup vs baseline: 1.0016x; 1.0016x over previous
"""Trainium2 Bass kernel for nn_CSFM_86011015070100 (topk_masking).

Data-parallel over batch: core b handles batch element b (B == 8 == n_cores).

Two launches per batch element:
  L1 (fused, rgb SBUF-resident):
    phase 1: stream rgb (kept resident) + ir; per-pixel channel-sum and
        channel-max via pair-op + PE transpose + DVE reduce; per-channel
        sum(x^2) partials on ScalarE.
    phase 2: 7x7 conv on device as 14 accumulating PE matmuls against
        host-precomputed banded weight matrices; double sigmoid via a
        degree-9 f32 exp polynomial + DVE reciprocal (~1 ulp).
    phase 3: per-channel dot(sa, x_c) partial sums (16-px blocks); rgb read
        from SBUF, ir re-streamed; sa broadcast to 128 partitions by a
        bit-exact fp32 ones-matmul on the otherwise idle PE.
  host: combine partials in f64 (exact sim ordering), stable argsort,
    positive counts, global k, gather tables.
  L2: indirect-DMA channel gather of rgb/ir + add -> output; host fixes up
    the single max-fused channel (when k_rgb != k_ir).
"""

import numpy as np
from contextlib import ExitStack

import concourse.bass as bass
import concourse.bacc as bacc
import concourse.tile as tile
from concourse import mybir
from concourse.bass_utils import run_bass_kernel_spmd
from concourse.masks import make_identity

F32 = mybir.dt.float32
I32 = mybir.dt.int32

B, C, H, W = 8, 256, 128, 128
HW = H * W          # 16384
NCORES = 8
CORE_IDS = list(range(NCORES))

PCH = 1024          # pixels per chunk in phase 1
NCH = HW // PCH     # 16
PCH3 = 2048         # pixels per chunk in phase 3
NCH3 = HW // PCH3   # 8
DBLK = 32           # pixels per dot partial-sum block
NDP = HW // DBLK    # 512 dot partials per channel
SBLK = PCH          # pixels per sum-of-squares partial block
NSP = HW // SBLK    # 16
GCHUNK = 4096       # pixels per gather chunk in L2
NGCH = HW // GCHUNK

# degree-9 minimax fit of exp(x) on [-1.5, 1.0]; ~5e-8 rel fit error
EXPC = (0.999999998645628, 0.9999999779969408, 0.5000000450179761,
        0.16666694288780706, 0.04166648306345862, 0.008332373827203384,
        0.001388966966072117, 0.00019966415132577638,
        2.5136363775979867e-05, 2.2273871202240795e-06)

_cache = {}

TRACE = False
LAST_EXEC_NS = []


def _run(nc, maps):
    try:
        r = run_bass_kernel_spmd(nc, maps, CORE_IDS, trace=TRACE)
    except Exception:
        import time

        time.sleep(2)
        r = run_bass_kernel_spmd(nc, maps, CORE_IDS, trace=TRACE)
    if r.exec_time_ns is not None:
        LAST_EXEC_NS.append(r.exec_time_ns)
    return r.results


# --------------------------------------------------------------------------
# L1: stats + on-device spatial attention + dot partials
# --------------------------------------------------------------------------
def _sigmoid_dev(nc, out, m, scratch):
    """out = 1/(1+exp(-m)) elementwise on a [128,128] tile, ~1 ulp f32."""
    vec = nc.vector
    u = scratch()
    t = scratch()
    acc = scratch()
    vec.tensor_scalar(out=u[:], in0=m, scalar1=-1.0, scalar2=None,
                      op0=mybir.AluOpType.mult)
    vec.memset(acc[:], float(EXPC[9]))
    for i in range(8, -1, -1):
        vec.tensor_tensor(out=t[:], in0=acc[:], in1=u[:],
                          op=mybir.AluOpType.mult)
        vec.tensor_scalar(out=acc[:], in0=t[:], scalar1=float(EXPC[i]),
                          scalar2=None, op0=mybir.AluOpType.add)
    vec.tensor_scalar(out=t[:], in0=acc[:], scalar1=1.0, scalar2=None,
                      op0=mybir.AluOpType.add)
    vec.reciprocal(out=out, in_=t[:])


def _build_l1():
    nc = bacc.Bacc("TRN2", target_bir_lowering=False, debug=False)
    rgb = nc.dram_tensor("rgb", [C, HW], F32, kind="ExternalInput").ap()
    ir = nc.dram_tensor("ir", [C, HW], F32, kind="ExternalInput").ap()
    # banded conv weights [128 w_in, 14 (plane*7+dh), 128 w_out]; avg-plane
    # bands pre-scaled by 1/256 on host
    wband = nc.dram_tensor("wband", [128, 14, 128], F32,
                           kind="ExternalInput").ap()
    bvec = nc.dram_tensor("bvec", [1, 1], F32, kind="ExternalInput").ap()
    dparts = nc.dram_tensor("dparts", [2, 2, 128, NDP], F32,
                            kind="ExternalOutput").ap()
    sparts = nc.dram_tensor("sparts", [2, 2, 128, NSP], F32,
                            kind="ExternalOutput").ap()
    sad = nc.dram_tensor("sad", [128, 128], F32, kind="Internal").ap()

    with tile.TileContext(nc) as tc, ExitStack() as ctx:
        consts = ctx.enter_context(tc.tile_pool(name="consts", bufs=1))
        # resident rgb: [128, 2 groups, 16384] = 128 KiB/partition
        rgbres = consts.tile([128, 2, HW], F32, name="rgbres")
        sp_acc = {}
        dp_acc = {}
        for t in range(2):
            for g in range(2):
                sp_acc[t, g] = consts.tile([128, NSP], F32, name=f"sp{t}{g}")
                dp_acc[t, g] = consts.tile([128, NDP], F32, name=f"dp{t}{g}")
        ones1 = consts.tile([1, 128], F32, name="ones1")
        nc.vector.memset(ones1[:], 1.0)

        # ---------------- phases 1+2 scoped tiles ----------------
        with tc.tile_pool(name="p12", bufs=1) as p12:
            ident = p12.tile([128, 128], F32, name="ident")
            make_identity(nc, ident[:])
            # stats planes [w, h] with 3-col zero padding each side (h -> 134)
            swh = {}
            for t in range(2):
                for kind in ("s", "m"):
                    p = p12.tile([128, 134], F32, name=f"swh_{kind}{t}")
                    nc.vector.memset(p[:], 0.0)
                    swh[kind, t] = p
            bsb = p12.tile([128, 1], F32, name="bsb")
            b_bcast = bass.AP(tensor=bvec.tensor, offset=bvec.offset,
                              ap=[[0, 128], [1, 1]])
            nc.sync.dma_start(out=bsb[:], in_=b_bcast)

            # ---------------- phase 1: stats ----------------
            with tc.tile_pool(name="ld1", bufs=4) as ld1, \
                 tc.tile_pool(name="sqd", bufs=2) as sqd, \
                 tc.tile_pool(name="pt1", bufs=2, space="PSUM") as pt1:
                for ci in range(NCH):
                    sl = slice(ci * PCH, (ci + 1) * PCH)
                    for t, x in enumerate((rgb, ir)):
                        xgs = []
                        for g in range(2):
                            if t == 0:
                                eng = nc.sync if g == 0 else nc.scalar
                                eng.dma_start(out=rgbres[:, g, sl],
                                              in_=x[g * 128:(g + 1) * 128, sl])
                                xgs.append(rgbres[:, g, sl])
                            else:
                                xt = ld1.tile([128, PCH], F32, tag="irl")
                                eng = nc.sync if g == 0 else nc.scalar
                                eng.dma_start(out=xt[:],
                                              in_=x[g * 128:(g + 1) * 128, sl])
                                xgs.append(xt[:])
                        # sumsq partials (ScalarE, full-chunk accumulate)
                        for g, xg in enumerate(xgs):
                            sq = sqd.tile([128, PCH], F32, tag="sq")
                            nc.scalar.activation(
                                out=sq[:], in_=xg,
                                func=mybir.ActivationFunctionType.Square,
                                accum_out=sp_acc[t, g][:, ci:ci + 1])
                        # pair ops 256 -> 128
                        gsum = ld1.tile([128, PCH], F32, tag="gsum")
                        nc.gpsimd.tensor_tensor(out=gsum[:], in0=xgs[0],
                                                in1=xgs[1],
                                                op=mybir.AluOpType.add)
                        gm = ld1.tile([128, PCH], F32, tag="gm")
                        nc.vector.tensor_tensor(out=gm[:], in0=xgs[0],
                                                in1=xgs[1],
                                                op=mybir.AluOpType.max)
                        # transpose 8 blocks of each into PSUM, one X-reduce
                        for kind, srct, op in (("s", gsum, mybir.AluOpType.add),
                                               ("m", gm, mybir.AluOpType.max)):
                            pt = pt1.tile([128, 8, 128], F32, tag=f"pt{kind}")
                            for b8 in range(8):
                                nc.tensor.transpose(
                                    pt[:, b8],
                                    srct[:, b8 * 128:(b8 + 1) * 128],
                                    ident[:])
                            nc.vector.tensor_reduce(
                                out=swh[kind, t][:, 3 + ci * 8:3 + ci * 8 + 8],
                                in_=pt[:], axis=mybir.AxisListType.X, op=op)

            # ---------------- phase 2: sa on device ----------------
            with tc.tile_pool(name="sas", bufs=1) as sas, \
                 tc.tile_pool(name="pt2", bufs=1, space="PSUM") as pt2:
                wb = sas.tile([128, 14, 128], F32, name="wb")
                nc.sync.dma_start(out=wb[:], in_=wband)
                _sc = [0]

                def scratch():
                    _sc[0] += 1
                    return sas.tile([128, 128], F32, name=f"scr{_sc[0]}")

                convs = []
                for t in range(2):
                    cp = pt2.tile([128, 128], F32, tag=f"conv{t}")
                    idx = 0
                    for plane, kind in ((0, "s"), (1, "m")):
                        for dh in range(7):
                            nc.tensor.matmul(
                                cp[:], wb[:, plane * 7 + dh],
                                swh[kind, t][:, dh:dh + 128],
                                start=(idx == 0), stop=(idx == 13))
                            idx += 1
                    cv = sas.tile([128, 128], F32, name=f"cv{t}")
                    nc.scalar.copy(out=cv[:], in_=cp[:])
                    convs.append(cv)
                m = sas.tile([128, 128], F32, name="mtile")
                nc.vector.tensor_tensor(out=m[:], in0=convs[0][:],
                                        in1=convs[1][:],
                                        op=mybir.AluOpType.max)
                nc.vector.tensor_scalar(out=m[:], in0=m[:],
                                        scalar1=bsb[:, 0:1], scalar2=None,
                                        op0=mybir.AluOpType.add)
                s1 = sas.tile([128, 128], F32, name="s1")
                _sigmoid_dev(nc, s1[:], m[:], scratch)
                sa_wh = sas.tile([128, 128], F32, name="sa_wh")
                _sigmoid_dev(nc, sa_wh[:], s1[:], scratch)
                # transpose [w,h] -> [h,w]; park in DRAM for phase 3
                pt = pt2.tile([128, 128], F32, tag="satp")
                nc.tensor.transpose(pt[:], sa_wh[:], ident[:])
                sa_hw = sas.tile([128, 128], F32, name="sa_hw")
                nc.scalar.copy(out=sa_hw[:], in_=pt[:])
                nc.scalar.dma_start(out=sad, in_=sa_hw[:])

        # ---------------- phase 3: dot partials ----------------
        with tc.tile_pool(name="ld3", bufs=2) as ld3, \
             tc.tile_pool(name="srp", bufs=2) as srp, \
             tc.tile_pool(name="sab", bufs=2) as sab, \
             tc.tile_pool(name="prodp", bufs=2) as prodp, \
             tc.tile_pool(name="pt3", bufs=2, space="PSUM") as pt3:
            for ci in range(NCH3):
                sl = slice(ci * PCH3, (ci + 1) * PCH3)
                # broadcast sa chunk to 128 partitions: exact fp32 ones-matmul
                ptb = pt3.tile([128, PCH3], F32, tag="ptb")
                for k in range(PCH3 // 512):
                    srow = srp.tile([1, 512], F32, tag="srow")
                    sa_rd = bass.AP(tensor=sad.tensor,
                                    offset=ci * PCH3 + k * 512,
                                    ap=[[512, 1], [1, 512]])
                    nc.scalar.dma_start(out=srow[:], in_=sa_rd)
                    nc.tensor.matmul(ptb[:, k * 512:(k + 1) * 512], ones1[:],
                                     srow[:], start=True, stop=True)
                sa128 = sab.tile([128, PCH3], F32, tag="sa128")
                nc.scalar.copy(out=sa128[:], in_=ptb[:])
                step = 0
                for t, x in enumerate((rgb, ir)):
                    for g in range(2):
                        if t == 0:
                            xg = rgbres[:, g, sl]
                        else:
                            xt = ld3.tile([128, PCH3], F32, tag="ir3")
                            eng = nc.sync if g == 0 else nc.scalar
                            eng.dma_start(out=xt[:],
                                          in_=x[g * 128:(g + 1) * 128, sl])
                            xg = xt[:]
                        prod = prodp.tile([128, PCH3], F32, tag="prod")
                        ngps = 3 if ci % 2 else 2
                        peng = nc.gpsimd if step < ngps else nc.vector
                        peng.tensor_tensor(out=prod[:], in0=xg, in1=sa128[:],
                                           op=mybir.AluOpType.mult)
                        nc.vector.tensor_reduce(
                            out=dp_acc[t, g][:, ci * (PCH3 // DBLK):(ci + 1) * (PCH3 // DBLK)],
                            in_=prod[:].rearrange("p (s q) -> p s q", q=DBLK),
                            axis=mybir.AxisListType.X, op=mybir.AluOpType.add)
                        step += 1
        for t in range(2):
            for g in range(2):
                nc.scalar.dma_start(out=dparts[t, g], in_=dp_acc[t, g][:])
                nc.scalar.dma_start(out=sparts[t, g], in_=sp_acc[t, g][:])

    nc.compile()
    return nc


# --------------------------------------------------------------------------
# L2: gather channels of rgb/ir by index and add
# --------------------------------------------------------------------------
def _build_l2():
    nc = bacc.Bacc("TRN2", target_bir_lowering=False, debug=False,
                   num_swdge_queues=2)
    rgb = nc.dram_tensor("rgb", [C, HW], F32, kind="ExternalInput").ap()
    ir = nc.dram_tensor("ir", [C, HW], F32, kind="ExternalInput").ap()
    gidx = nc.dram_tensor("gidx", [2, C], I32, kind="ExternalInput").ap()
    out = nc.dram_tensor("out", [C, HW], F32, kind="ExternalOutput").ap()

    with tile.TileContext(nc) as tc, ExitStack() as ctx:
        idxp = ctx.enter_context(tc.tile_pool(name="idxp", bufs=1))
        rp = ctx.enter_context(tc.tile_pool(name="rp", bufs=3))
        ip = ctx.enter_context(tc.tile_pool(name="ip", bufs=3))
        op = ctx.enter_context(tc.tile_pool(name="op", bufs=3))

        for g in range(2):
            idr = idxp.tile([128, 1], I32, tag=f"idr{g}")
            idi = idxp.tile([128, 1], I32, tag=f"idi{g}")
            nc.sync.dma_start(out=idr[:], in_=gidx[0, g * 128:(g + 1) * 128])
            nc.sync.dma_start(out=idi[:], in_=gidx[1, g * 128:(g + 1) * 128])
            for ci in range(NGCH):
                sl = slice(ci * GCHUNK, (ci + 1) * GCHUNK)
                rt = rp.tile([128, GCHUNK], F32, tag="rt")
                it = ip.tile([128, GCHUNK], F32, tag="it")
                nc.gpsimd.indirect_dma_start(
                    out=rt[:], out_offset=None, in_=rgb,
                    in_offset=bass.IndirectOffsetOnAxis(ap=idr[:, 0:1], axis=0),
                    element_offset=ci * GCHUNK)
                inst = nc.gpsimd.indirect_dma_start(
                    out=it[:], out_offset=None, in_=ir,
                    in_offset=bass.IndirectOffsetOnAxis(ap=idi[:, 0:1], axis=0),
                    element_offset=ci * GCHUNK)
                inst.ins.queue = "qPoolDynamic1"
                ot = op.tile([128, GCHUNK], F32, tag="ot")
                nc.vector.tensor_tensor(out=ot[:], in0=rt[:], in1=it[:],
                                        op=mybir.AluOpType.add)
                nc.sync.dma_start(out=out[g * 128:(g + 1) * 128, sl], in_=ot[:])

    nc.compile()
    return nc


def _get(name, builder):
    if name not in _cache:
        _cache[name] = builder()
    return _cache[name]


# --------------------------------------------------------------------------
# host glue
# --------------------------------------------------------------------------
def _make_wband(conv_w):
    """[128 w_in, 14 (plane*7+dh), 128 w_out]: W[wi, k, wo] = cw[plane,dh,dw]
    at wi = wo+dw-3; avg-plane entries pre-scaled by 1/256."""
    cw = np.asarray(conv_w, dtype=np.float64)  # [1,2,7,7]
    wb = np.zeros((14, 128, 128), np.float64)
    for plane in range(2):
        scale = (1.0 / 256.0) if plane == 0 else 1.0
        for dh in range(7):
            for dw in range(7):
                v = cw[0, plane, dh, dw] * scale
                wi0 = max(0, 3 - dw)
                wi = np.arange(128) + dw - 3
                ok = (wi >= 0) & (wi < 128)
                wb[plane * 7 + dh, wi[ok], np.arange(128)[ok]] = v
    return np.ascontiguousarray(wb.transpose(1, 0, 2)).astype(np.float32)


def kernel(rgb, ir, conv_w, conv_b):
    rgb = np.ascontiguousarray(rgb, dtype=np.float32)
    ir = np.ascontiguousarray(ir, dtype=np.float32)
    conv_w = np.asarray(conv_w, dtype=np.float32)
    conv_b = np.asarray(conv_b, dtype=np.float32)

    rgb2 = rgb.reshape(B, C, HW)
    ir2 = ir.reshape(B, C, HW)
    LAST_EXEC_NS.clear()

    wband = _make_wband(conv_w)
    bvec = conv_b.reshape(1, 1)

    # ---- L1
    nc1 = _get("l1", _build_l1)
    maps1 = [{"rgb": rgb2[b], "ir": ir2[b], "wband": wband, "bvec": bvec}
             for b in range(B)]
    res1 = _run(nc1, maps1)

    # ---- host: sims, orders, counts, tables (f64 combine of partials)
    orders = np.zeros((B, 2, C), np.int64)
    cnts = np.zeros((B, 2), np.int64)
    for b in range(B):
        dparts = res1[b]["dparts"].astype(np.float64)  # [2,2,128,NDP]
        sparts = res1[b]["sparts"].astype(np.float64)  # [2,2,128,NSP]
        for t in range(2):
            dot = np.concatenate([dparts[t, 0].sum(-1), dparts[t, 1].sum(-1)])
            sq = np.concatenate([sparts[t, 0].sum(-1), sparts[t, 1].sum(-1)])
            tv = dot / np.maximum(np.sqrt(sq), 1e-30)
            orders[b, t] = np.argsort(tv, kind="stable")
            cnts[b, t] = int((tv > 0).sum())
    k_rgb = int(cnts[:, 0].max())
    k_ir = int(cnts[:, 1].max())
    ch = np.arange(C)
    src_rgb = ch.copy()
    src_ir = ch.copy()
    if k_rgb < k_ir:
        src_rgb[ch > k_rgb] -= 1
    elif k_ir < k_rgb:
        src_ir[ch > k_ir] -= 1

    # ---- L2
    nc2 = _get("l2", _build_l2)
    gidxs = []
    for b in range(B):
        g_r = orders[b, 0][src_rgb]
        g_i = orders[b, 1][src_ir]
        gidxs.append(np.stack([g_r, g_i]).astype(np.int32))
    maps2 = [{"rgb": rgb2[b], "ir": ir2[b], "gidx": gidxs[b]} for b in range(B)]
    res2 = _run(nc2, maps2)
    out = np.stack([res2[b]["out"].reshape(C, H, W) for b in range(B)])

    # ---- host fixup of the max-fused channel
    if k_rgb != k_ir:
        kpos = min(k_rgb, k_ir)
        for b in range(B):
            maxfea = np.maximum(rgb2[b, orders[b, 0][0]], ir2[b, orders[b, 1][0]])
            if k_rgb < k_ir:
                other = ir2[b, gidxs[b][1][kpos]]
            else:
                other = rgb2[b, gidxs[b][0][kpos]]
            out[b, kpos] = (maxfea + other).reshape(H, W)

    return out
